# revision 1
# baseline (speedup 1.0000x reference)
"""MoE top-2 routing + expert FFN for Trainium2, expert-parallel across 8 cores.

Contract: kernel(**inputs) takes the FULL unsharded inputs (numpy) and
returns the FULL output [B, S, D] float32.

Strategy:
  - Host computes the router (softmax -> top-2 -> renormalize) with the exact
    same jax-on-CPU ops as the reference, so expert selection is bit-identical.
  - Tokens are gathered per expert; core c runs expert c's FFN over its tokens
    (padded to a shared capacity C computed from the actual counts).
  - Device computes, in feature-major layout (no transposes anywhere):
        H1 = gelu(W1^T @ X^T + b1)   [H, C]
        Y^T = W2^T @ H1 + b2         [D, C]
    with float32r matmuls (full PE rate, FP22 multiply, FP32 accumulate).
  - Host scales each row by its gate weight and scatter-adds into the output.
"""

import numpy as np
from contextlib import ExitStack

B, S, D = 4, 2048, 1024
E, H, TOP_K = 8, 4096, 2
T = B * S
P = 128
KS1 = D // P          # 8  k-subtiles for the first matmul
HH = 2                # H halves
HML = H // HH // P    # 16 h-chunks per half
DC = D // P           # 8  output d-chunks


def _routing(xf, Wr, br):
    """Bit-identical replication of the reference's routing on jax CPU."""
    import jax
    import jax.numpy as jnp

    cpu = jax.local_devices(backend="cpu")[0]
    with jax.default_device(cpu):
        gate = jax.nn.softmax(jnp.asarray(xf) @ jnp.asarray(Wr) + jnp.asarray(br), axis=-1)
        top_w, top_i = jax.lax.top_k(gate, TOP_K)
        top_w = top_w / jnp.sum(top_w, axis=-1, keepdims=True)
    return np.asarray(top_i), np.asarray(top_w)


def _pieces(TB):
    """Split TB into free-dim pieces, each <=512 (one PSUM bank) and >=256
    (float32r full-rate threshold) when possible."""
    if TB <= 512:
        return [(0, TB)]
    if TB % 2 == 0 and 256 <= TB // 2 <= 512:
        h = TB // 2
        return [(0, h), (h, h)]
    out, off, rem = [], 0, TB
    while rem:
        take = min(512, rem)
        if rem - take and rem - take < 256:
            take = rem - 256
        out.append((off, take))
        off += take
        rem -= take
    return out


def _blocks(C):
    """Split C into <=512 pieces (>=256 each), grouped into blocks of <=2
    pieces (block width <=1024, bounding the H1/XT/Y tile sizes)."""
    pieces, rem = [], C
    while rem:
        take = min(512, rem)
        if rem - take and rem - take < 256:
            take = (rem // 2) // 4 * 4
        pieces.append(take)
        rem -= take
    blocks, t0 = [], 0
    while pieces:
        grp = pieces[:2] if len(pieces) >= 2 and pieces[0] + pieces[1] <= 1024 else pieces[:1]
        pieces = pieces[len(grp):]
        offs, bp = 0, []
        for pn in grp:
            bp.append((offs, pn))
            offs += pn
        blocks.append((t0, offs, bp))
        t0 += offs
    return blocks


def _build_program(C, TB, repeats):
    import concourse.tile as tile
    from concourse import bacc, mybir

    F32 = mybir.dt.float32
    F32R = mybir.dt.float32r
    blocks = _blocks(C)

    nc = bacc.Bacc("TRN2", target_bir_lowering=False, debug=False, num_devices=E)

    xt_ap = nc.dram_tensor("xt", [P, KS1, C], F32, kind="ExternalInput").ap()
    w1_ap = nc.dram_tensor("w1", [HH, HML, P, KS1, P], F32, kind="ExternalInput").ap()
    w2_ap = nc.dram_tensor("w2", [HH, DC, P, HML, P], F32, kind="ExternalInput").ap()
    b1_ap = nc.dram_tensor("b1", [HH, HML, P], F32, kind="ExternalInput").ap()
    b2_ap = nc.dram_tensor("b2", [DC, P], F32, kind="ExternalInput").ap()
    y_ap = nc.dram_tensor("y", [P, DC, C], F32, kind="ExternalOutput").ap()

    with tile.TileContext(nc) as tc, ExitStack() as ctx:
        xt_pool = ctx.enter_context(tc.tile_pool(name="xt", bufs=1))
        h1_pool = ctx.enter_context(tc.tile_pool(name="h1", bufs=1))
        y_pool = ctx.enter_context(tc.tile_pool(name="y", bufs=1))
        w1_pool = ctx.enter_context(tc.tile_pool(name="w1", bufs=4))
        w2_pool = ctx.enter_context(tc.tile_pool(name="w2", bufs=4))
        bias_pool = ctx.enter_context(tc.tile_pool(name="bias", bufs=1))
        psA = ctx.enter_context(tc.tile_pool(name="psA", bufs=3, space="PSUM"))
        psB = ctx.enter_context(tc.tile_pool(name="psB", bufs=3, space="PSUM"))

        b1t = bias_pool.tile([P, HH * HML], F32)
        nc.sync.dma_start(b1t[:], b1_ap.rearrange("hh m p -> p (hh m)"))
        b2t = bias_pool.tile([P, DC], F32)
        nc.sync.dma_start(b2t[:], b2_ap.rearrange("d p -> p d"))

        def body():
            for t0, TBb, bpieces in blocks:
                xt = xt_pool.tile([P, KS1, TBb], F32R, tag="xt", name="xt")
                for po, pn in bpieces:
                    nc.sync.dma_start(
                        xt[:, :, po : po + pn],
                        xt_ap[:, :, t0 + po : t0 + po + pn].bitcast(F32R),
                    )
                yt = y_pool.tile([P, DC, TBb], F32, tag="y", name="yt")
                for hh in range(HH):
                    h1 = h1_pool.tile([P, HML, TBb], F32R, tag="h1", name="h1")
                    for m in range(HML):
                        w1t = w1_pool.tile([P, KS1, P], F32R, tag="w1", name="w1t")
                        nc.sync.dma_start(w1t[:], w1_ap[hh, m].bitcast(F32R))
                        for po, pn in bpieces:
                            ps = psA.tile([P, 512], F32, tag="psA", name="psA")[:, :pn]
                            for k in range(KS1):
                                nc.tensor.matmul(
                                    ps,
                                    w1t[:, k, :],
                                    xt[:, k, po : po + pn],
                                    start=(k == 0),
                                    stop=(k == KS1 - 1),
                                )
                            nc.scalar.activation(
                                h1[:, m, po : po + pn],
                                ps,
                                mybir.ActivationFunctionType.Gelu,
                                bias=b1t[:, hh * HML + m : hh * HML + m + 1],
                            )
                    for d in range(DC):
                        w2t = w2_pool.tile([P, HML, P], F32R, tag="w2", name="w2t")
                        nc.sync.dma_start(w2t[:], w2_ap[hh, d].bitcast(F32R))
                        for po, pn in bpieces:
                            ps = psB.tile([P, 512], F32, tag="psB", name="psB")[:, :pn]
                            for k in range(HML):
                                nc.tensor.matmul(
                                    ps,
                                    w2t[:, k, :],
                                    h1[:, k, po : po + pn],
                                    start=(k == 0),
                                    stop=(k == HML - 1),
                                )
                            if hh == 0:
                                nc.vector.tensor_tensor(
                                    yt[:, d, po : po + pn],
                                    ps,
                                    b2t[:, d : d + 1].to_broadcast([P, pn]),
                                    mybir.AluOpType.add,
                                )
                            else:
                                nc.vector.tensor_tensor(
                                    yt[:, d, po : po + pn],
                                    ps,
                                    yt[:, d, po : po + pn],
                                    mybir.AluOpType.add,
                                )
                nc.sync.dma_start(y_ap[:, :, t0 : t0 + TBb], yt[:])

        if repeats > 1:
            with tc.For_i(0, repeats, 1):
                body()
        else:
            body()

    nc.finalize()
    return nc


def _pack_inputs(xf, W1, b1, W2, b2, top_i, top_w, C, TB):
    """Per-expert gather + weight prepack into the device tile layouts."""
    NTB = C // TB
    in_maps = []
    idx_list = []
    w_list = []
    for e in range(E):
        sel = (top_i == e).any(axis=1)
        idx = np.nonzero(sel)[0]
        we = (top_w * (top_i == e)).sum(axis=1)[idx].astype(np.float32)
        idx_list.append(idx)
        w_list.append(we)

        n = len(idx)
        Xg = np.zeros((C, D), dtype=np.float32)
        Xg[:n] = xf[idx]
        # [NTB, P, KS1, TB]: token block tb, partition p = d % 128, k-subtile
        # ks = d // 128 -- contiguous per-partition runs for the device DMA.
        xt = np.ascontiguousarray(Xg.reshape(C, KS1, P).transpose(2, 1, 0))

        w1p = np.ascontiguousarray(
            W1[e].reshape(KS1, P, HH, HML, P).transpose(2, 3, 1, 0, 4)
        )
        w2p = np.ascontiguousarray(
            W2[e].reshape(HH, HML, P, DC, P).transpose(0, 3, 2, 1, 4)
        )
        b1p = np.ascontiguousarray(b1[e].reshape(HH, HML, P))
        b2p = np.ascontiguousarray(b2[e].reshape(DC, P))

        in_maps.append({"xt": xt, "w1": w1p, "w2": w2p, "b1": b1p, "b2": b2p})
    return in_maps, idx_list, w_list


def _run(x, Wr, br, W1, b1, W2, b2, repeats=1, timing_runs=0):
    import time

    from concourse.bass_utils import run_bass_kernel_spmd

    x = np.asarray(x, dtype=np.float32)
    Wr = np.asarray(Wr, dtype=np.float32)
    br = np.asarray(br, dtype=np.float32)
    W1 = np.asarray(W1, dtype=np.float32)
    b1 = np.asarray(b1, dtype=np.float32)
    W2 = np.asarray(W2, dtype=np.float32)
    b2 = np.asarray(b2, dtype=np.float32)

    xf = x.reshape(T, D)
    top_i, top_w = _routing(xf, Wr, br)

    counts = np.bincount(top_i.ravel(), minlength=E)
    max_count = int(counts.max())
    TB = max(256, 8 * int(np.ceil(max_count / (3 * 8))))
    C = 3 * TB

    nc = _build_program(C, TB, repeats)
    in_maps, idx_list, w_list = _pack_inputs(xf, W1, b1, W2, b2, top_i, top_w, C, TB)

    res = run_bass_kernel_spmd(nc, in_maps, core_ids=list(range(E)))

    walls = []
    for _ in range(timing_runs):
        t0 = time.perf_counter()
        run_bass_kernel_spmd(nc, in_maps, core_ids=list(range(E)))
        walls.append(time.perf_counter() - t0)

    out = np.zeros((T, D), dtype=np.float32)
    for e in range(E):
        idx = idx_list[e]
        n = len(idx)
        if n == 0:
            continue
        # y is [P, DC, C] with d = dc * 128 + p
        yp = res.results[e]["y"]
        Ye = yp.transpose(2, 1, 0).reshape(-1, D)  # [C, D]
        out[idx] += w_list[e][:, None] * Ye[:n]

    return out.reshape(B, S, D), walls


def kernel(x, Wr, br, W1, b1, W2, b2):
    out, _ = _run(x, Wr, br, W1, b1, W2, b2, repeats=1)
    return out

